# revision 15
# baseline (speedup 1.0000x reference)
"""Trainium2 Bass kernels for DGCNN-style edge-conv block (gnn_message_passing).

Math (per batch b):
  f1 = W1 f; f2 = W2 f  (biases provably cancel: per-channel constants pass
      through the k-max and are removed by train-mode BN)
  x  = max_k f1[:, idx[n,k]] + max_k f2[:, idx[n,k]] + (W3 - W1) f
  out = BN(x) over (B, N) per channel, gamma/beta affine.

Sharding: data-parallel over B (1 point cloud per core, 8 cores).

Cross-core BN stats: the device collective hangs under the axon PJRT path,
so this uses two launches: kernel A produces x (bf16) and per-core [128,2]
partial stats; the host reduces the tiny stat tensors and computes
scale/bias; kernel B applies the affine normalization.

Kernel A layout tricks (all verified on HW):
  * f1/f2 are packed as a (bf16,bf16) pair into one u32 word per channel:
    g[p, n] = pack(f1[p], f2[p]) on partitions 0-63, replicated to 64-127.
    One ap_gather word then carries BOTH streams of a channel, and the two
    partition halves gather DIFFERENT point chunks (all 8 gpsimd cores do
    independent work) -> 4x fewer gather-slots than the unpacked layout.
  * ap_gather's HW cost scales with num_idxs only; the source AP is passed
    as a narrow window at column 0 (the ucode addresses src by base +
    index, num_elems is the real extent) so small double-buffered gather
    tiles pipeline perfectly against the DVE k-max.
  * k-max is a 4-round pairwise-max tree on DVE in bf16 (2x mode): round 1
    into t1, rounds 2-4 in place; then pair-sum + h-add fused with the BN
    sum accumulation (scalar_tensor_tensor accum_out).
  * h = (W3-W1) f is computed in phase 2 by PE (idle there) with
    zero-padded stationaries [w31|0], [0|w31] so the two point chunks land
    on the two PSUM partition halves.
  * DMA dispatch (HWDGE) is serialized at ~625ns/transfer, so transfers
    are batched: big f/idx chunks, x written every second call, replicas
    copied at 4096-column granularity.
"""

import numpy as np
import ml_dtypes

import concourse.bass as bass
import concourse.bacc as bacc
import concourse.mybir as mybir
import concourse.tile as tile
from concourse import bass_utils

F32 = mybir.dt.float32
BF16 = mybir.dt.bfloat16
U32 = mybir.dt.uint32
I16 = mybir.dt.int16
ALU = mybir.AluOpType
BF = ml_dtypes.bfloat16

C = 64          # channels
FULL_N = 24576  # points per cloud
FULL_B = 8      # batches == cores
K = 16          # neighbors
P = 384         # points per partition-half per gather call
BN_EPS = 1e-5


def build_kernel_a(n_cores=FULL_B, N=FULL_N):
    NCALLS = N // (2 * P)    # 32
    CALLS = [P] * (NCALLS - 1) + [P // 2, P // 2]
    assert sum(CALLS) == N // 2
    FCH = 3072               # phase-1 f-load chunk (big: DMA latency ~3us)
    RCH = 4096               # replicate chunk
    IDXCH = 4                # calls per idx DMA
    FBCH = 4                 # calls per f2 DMA
    nc = bacc.Bacc("TRN2", target_bir_lowering=False, debug=False,
                   num_devices=n_cores)

    f_d = nc.dram_tensor("f", [C, N], BF16, kind="ExternalInput")
    idxw_d = nc.dram_tensor("idxw", [128, N // 2], I16, kind="ExternalInput")
    wcat_d = nc.dram_tensor("wcat", [C, 128], BF16, kind="ExternalInput")
    wpa_d = nc.dram_tensor("wpa", [C, 128], BF16, kind="ExternalInput")
    wpb_d = nc.dram_tensor("wpb", [C, 128], BF16, kind="ExternalInput")
    x_d = nc.dram_tensor("xout", [128, N // 2], BF16, kind="ExternalOutput")
    st_d = nc.dram_tensor("stats", [128, 2], F32, kind="ExternalOutput")

    with tile.TileContext(nc) as tc:
        with (
            tc.tile_pool(name="const", bufs=1) as constp,
            tc.tile_pool(name="gpool", bufs=1) as gpool,
            tc.tile_pool(name="stat", bufs=1) as statp,
            tc.tile_pool(name="fio", bufs=2) as fio,
            tc.tile_pool(name="gath", bufs=2) as gath,
            tc.tile_pool(name="tree", bufs=1) as tree,
            tc.tile_pool(name="work", bufs=2) as work,
            tc.tile_pool(name="ps1", bufs=3, space="PSUM") as ps1,
            tc.tile_pool(name="ps2", bufs=2, space="PSUM") as ps2,
        ):
            g = gpool.tile([128, N], U32)
            ft0 = fio.tile([C, FCH], BF16, tag="ft")
            nc.sync.dma_start(ft0[:], f_d.ap()[:, 0:FCH])
            wct = constp.tile([C, 128], BF16)
            wpa = constp.tile([C, 128], BF16)
            wpb = constp.tile([C, 128], BF16)
            nc.sync.dma_start(wct[:], wcat_d.ap())
            nc.sync.dma_start(wpa[:], wpa_d.ap())
            nc.sync.dma_start(wpb[:], wpb_d.ap())

            # ---- phase 1: g[p,n] = pack(f1,f2) on the low half (DVE even
            # lane, ACT odd lane), then DMA-replicate low -> high.
            for c0 in range(0, N, FCH):
                if c0 == 0:
                    ft = ft0
                else:
                    ft = fio.tile([C, FCH], BF16, tag="ft")
                    nc.sync.dma_start(ft[:], f_d.ap()[:, c0:c0 + FCH])
                for s0 in range(0, FCH, 512):
                    col = c0 + s0
                    gp = ps1.tile([128, 512], F32, tag="gps")
                    nc.tensor.matmul(gp[:], wct[:], ft[:, s0:s0 + 512],
                                     start=True, stop=True)
                    gb = g[0:C, col:col + 512].bitcast(BF16).rearrange(
                        "p (n t) -> p n t", t=2)
                    nc.vector.tensor_copy(gb[:, :, 0:1].squeeze(2),
                                          gp[0:C, :])
                    nc.scalar.copy(gb[:, :, 1:2].squeeze(2), gp[C:128, :])
                    end = col + 512
                    if end % RCH == 0 and end < N:
                        nc.scalar.dma_start(g[C:128, end - RCH:end],
                                            g[0:C, end - RCH:end])
                    elif end == N:
                        nc.scalar.dma_start(g[C:128, N - RCH:N - RCH // 2],
                                            g[0:C, N - RCH:N - RCH // 2])
                        nc.scalar.dma_start(g[C:128, N - RCH // 2:N],
                                            g[0:C, N - RCH // 2:N])

            # Order all later Pool gathers after every write to g. One tiny
            # contiguous gpsimd read at the tail of each replicate block:
            # its low half depends on that block's last pack copies (DVE/ACT
            # are in-order, covering all earlier chunks), its high half on
    # that block's replicate DMA. In-order Pool then sequences all
            # gathers after these fences, while phase-2 input DMAs are free
            # to prefetch during phase 1 (a strict barrier would block them).
            fpts = list(range(RCH, N + 1, RCH)) + [N - RCH // 2]
            fence = statp.tile([128, 4 * len(fpts)], U32)
            for i, r in enumerate(fpts):
                nc.gpsimd.tensor_copy(fence[:, 4 * i:4 * i + 4],
                                      g[:, r - 4:r])
            # scheduler-only fence: keeps the gathers (whose narrow source
            # APs carry no dep on most of g) behind the fence reads without
            # synthesizing semaphore waits for phase-2 input DMAs.
            tc.no_sync_barrier()

            # ---- phase 2: gather + k-max + h -> x tiles, stat partials.
            # The last two calls are half-size so the serial DVE tree tail
            # after the final gather is short.
            scol = statp.tile([128, len(CALLS)], F32)
            qcol = statp.tile([128, len(CALLS)], F32)

            c0 = 0
            for q, Pq in enumerate(CALLS):
                if c0 % (IDXCH * P) == 0:
                    it = work.tile([128, IDXCH * P], I16, tag="idx")
                    nc.sync.dma_start(
                        it[:], idxw_d.ap()[:, c0:c0 + IDXCH * P])
                if (2 * c0) % (FBCH * 2 * P) == 0:
                    f2t = work.tile([C, FBCH * 2 * P], BF16, tag="f2")
                    nc.sync.dma_start(
                        f2t[:], f_d.ap()[:, 2 * c0:2 * c0 + FBCH * 2 * P])
                iq = c0 % (IDXCH * P)
                fq = (2 * c0) % (FBCH * 2 * P)

                gt = gath.tile([128, 16 * P], U32, tag="gt")
                nc.gpsimd.ap_gather(gt[:, :16 * Pq], g[:, 0:4],
                                    it[:, iq:iq + Pq], channels=128,
                                    num_elems=N, d=1, num_idxs=16 * Pq)

                hp = ps2.tile([128, P], F32, tag="hp")
                nc.tensor.matmul(hp[:, :Pq], wpa[:], f2t[:, fq:fq + Pq],
                                 start=True, stop=False)
                nc.tensor.matmul(hp[:, :Pq], wpb[:],
                                 f2t[:, fq + Pq:fq + 2 * Pq],
                                 start=False, stop=True)

                gtb = gt[:, :16 * Pq].bitcast(BF16).rearrange(
                    "p (n k t) -> p n k t", k=K, t=2)
                t1 = tree.tile([128, P * K], BF16, tag="t1")
                t1v = t1[:, :Pq * K].rearrange(
                    "p (n k t) -> p n k t", k=8, t=2)
                nc.vector.tensor_tensor(
                    t1v, gtb[:, :, 0:8, :], gtb[:, :, 8:16, :], ALU.max)
                nc.vector.tensor_tensor(
                    t1v[:, :, 0:4, :], t1v[:, :, 0:4, :], t1v[:, :, 4:8, :],
                    ALU.max)
                nc.vector.tensor_tensor(
                    t1v[:, :, 0:2, :], t1v[:, :, 0:2, :], t1v[:, :, 2:4, :],
                    ALU.max)
                nc.vector.tensor_tensor(
                    t1v[:, :, 0:1, :], t1v[:, :, 0:1, :], t1v[:, :, 1:2, :],
                    ALU.max)
                m1 = t1v[:, :, 0, 0]
                m2 = t1v[:, :, 0, 1]
                tmp = work.tile([128, P], F32, tag="tmp")
                nc.vector.tensor_tensor(tmp[:, :Pq], m1, m2, ALU.add)
                xt = work.tile([128, P], BF16, tag="xt")
                nc.vector.scalar_tensor_tensor(
                    xt[:, :Pq], tmp[:, :Pq], 1.0, hp[:, :Pq],
                    ALU.mult, ALU.add, accum_out=scol[:, q:q + 1])
                sq = work.tile([128, P], BF16, tag="sq")
                nc.scalar.activation(
                    out=sq[:, :Pq], in_=xt[:, :Pq],
                    func=mybir.ActivationFunctionType.Square,
                    accum_out=qcol[:, q:q + 1])
                nc.scalar.dma_start(x_d.ap()[:, c0:c0 + Pq], xt[:, :Pq])
                c0 += Pq

            # ---- phase 3: per-core stat partials out ----
            pair = statp.tile([128, 2], F32)
            nc.vector.tensor_reduce(pair[:, 0:1], scol[:],
                                    axis=mybir.AxisListType.X, op=ALU.add)
            nc.vector.tensor_reduce(pair[:, 1:2], qcol[:],
                                    axis=mybir.AxisListType.X, op=ALU.add)
            nc.scalar.dma_start(st_d.ap(), pair[:])

    nc.compile()
    return nc


def build_kernel_b(n_cores=FULL_B, N=FULL_N):
    """out = x * scale[p] + bias[p]; x is the [128, N/2] stacked-chunk bf16."""
    NH = N // 2
    nc = bacc.Bacc("TRN2", target_bir_lowering=False, debug=False,
                   num_devices=n_cores)
    x_d = nc.dram_tensor("xout", [128, NH], BF16, kind="ExternalInput")
    scb_d = nc.dram_tensor("scb", [128, 2], F32, kind="ExternalInput")
    out_d = nc.dram_tensor("out", [128, NH], BF16, kind="ExternalOutput")
    CH = 2048
    with tile.TileContext(nc) as tc:
        with (
            tc.tile_pool(name="const", bufs=1) as constp,
            tc.tile_pool(name="io", bufs=6) as io,
        ):
            scb = constp.tile([128, 2], F32)
            nc.sync.dma_start(scb[:], scb_d.ap())
            for i, c0 in enumerate(range(0, NH, CH)):
                w = min(CH, NH - c0)
                t = io.tile([128, CH], BF16, tag="xin")
                nc.sync.dma_start(t[:, :w], x_d.ap()[:, c0:c0 + w])
                o = io.tile([128, CH], BF16, tag="xo")
                if i % 2 == 0:
                    nc.scalar.activation(
                        out=o[:, :w], in_=t[:, :w],
                        func=mybir.ActivationFunctionType.Identity,
                        bias=scb[:, 1:2], scale=scb[:, 0:1])
                else:
                    nc.vector.tensor_scalar(
                        out=o[:, :w], in0=t[:, :w],
                        scalar1=scb[:, 0:1], scalar2=scb[:, 1:2],
                        op0=ALU.mult, op1=ALU.add)
                nc.sync.dma_start(out_d.ap()[:, c0:c0 + w], o[:, :w])
    nc.compile()
    return nc


def prep_inputs_a(f, idx, W1, W2, W3, n_cores, N):
    w31 = (W3.astype(np.float64) - W1.astype(np.float64))
    wcat = np.vstack([W1, W2]).T.astype(BF)                  # [64, 128]
    zero = np.zeros((C, C), np.float64)
    wpa = np.hstack([w31.T, zero]).astype(BF)                # [64, 128]
    wpb = np.hstack([zero, w31.T]).astype(BF)
    calls = call_sizes(N)
    in_maps = []
    for b in range(n_cores):
        # wrapped index layout: call q occupies Pq columns; rows 0-63 get
        # chunk A (points [2c0, 2c0+Pq)) x4, rows 64-127 chunk B x4.
        ib = idx[b].astype(np.int16)
        iw = np.empty((128, N // 2), np.int16)
        c0 = 0
        for Pq in calls:
            a = ib[2 * c0:2 * c0 + Pq].T                     # [K, Pq]
            bb = ib[2 * c0 + Pq:2 * c0 + 2 * Pq].T
            iw[0:64, c0:c0 + Pq] = np.tile(a, (4, 1))
            iw[64:128, c0:c0 + Pq] = np.tile(bb, (4, 1))
            c0 += Pq
        in_maps.append({
            "f": np.ascontiguousarray(f[b]).astype(BF),
            "idxw": np.ascontiguousarray(iw),
            "wcat": np.ascontiguousarray(wcat),
            "wpa": np.ascontiguousarray(wpa),
            "wpb": np.ascontiguousarray(wpb),
        })
    return in_maps


def host_scale_bias(stats, gamma, beta, total_cnt):
    """stats: [B, 128, 2] per-core partial (sum, sumsq) -> scb [128, 2]."""
    tot = stats.astype(np.float64).sum(axis=0)     # [128, 2]
    tot = tot[0:C] + tot[C:128]                    # fold partition halves
    mean = tot[:, 0] / total_cnt
    var = tot[:, 1] / total_cnt - mean * mean
    rstd = 1.0 / np.sqrt(var + BN_EPS)
    scale = np.asarray(gamma, np.float64) * rstd
    bias = np.asarray(beta, np.float64) - mean * scale
    scb = np.stack([scale, bias], axis=1).astype(np.float32)  # [64, 2]
    return np.tile(scb, (2, 1)).astype(np.float32)            # [128, 2]


def call_sizes(N):
    ncalls = N // (2 * P)
    return [P] * (ncalls - 1) + [P // 2, P // 2]


def unshard_x(xo, N):
    """[128, N/2] stacked-chunk layout -> [64, N] channels-major."""
    x = np.empty((C, N), xo.dtype)
    c0 = 0
    for Pq in call_sizes(N):
        x[:, 2 * c0:2 * c0 + Pq] = xo[0:64, c0:c0 + Pq]
        x[:, 2 * c0 + Pq:2 * c0 + 2 * Pq] = xo[64:128, c0:c0 + Pq]
        c0 += Pq
    return np.ascontiguousarray(x)


_NC_CACHE = {}


def kernel(f, idx, W1, b1, W2, b2, W3, b3, gamma, beta):
    f = np.asarray(f)
    idx = np.asarray(idx)
    B, C_, N = f.shape
    key = (B, N)
    if key not in _NC_CACHE:
        _NC_CACHE[key] = (build_kernel_a(n_cores=B, N=N),
                          build_kernel_b(n_cores=B, N=N))
    nca, ncb = _NC_CACHE[key]
    in_maps = prep_inputs_a(f, idx, np.asarray(W1), np.asarray(W2),
                            np.asarray(W3), B, N)
    res_a = bass_utils.run_bass_kernel_spmd(nca, in_maps,
                                            core_ids=list(range(B)))
    stats = np.stack([res_a.results[b]["stats"] for b in range(B)])
    scb = host_scale_bias(stats, gamma, beta, B * N)
    in_maps_b = [{"xout": res_a.results[b]["xout"].view(BF), "scb": scb}
                 for b in range(B)]
    res_b = bass_utils.run_bass_kernel_spmd(ncb, in_maps_b,
                                            core_ids=list(range(B)))
    out = np.stack([unshard_x(res_b.results[b]["out"].view(BF), N)
                    for b in range(B)], axis=0)
    kernel.last_results = (res_a, res_b)
    return out.astype(np.float32)


# revision 24
# speedup vs baseline: 1.0189x; 1.0189x over previous
"""Trainium2 Bass kernels for DGCNN-style edge-conv block (gnn_message_passing).

Math (per batch b):
  f1 = W1 f; f2 = W2 f  (biases provably cancel: per-channel constants pass
      through the k-max and are removed by train-mode BN)
  x  = max_k f1[:, idx[n,k]] + max_k f2[:, idx[n,k]] + (W3 - W1) f
  out = BN(x) over (B, N) per channel, gamma/beta affine.

Sharding: data-parallel over B (1 point cloud per core, 8 cores).

Cross-core BN stats: the device collective hangs under the axon PJRT path,
so this uses two launches: kernel A produces x (bf16) and per-core [128,2]
partial stats; the host reduces the tiny stat tensors and computes
scale/bias; kernel B applies the affine normalization.

Kernel A layout tricks (all verified on HW):
  * f1/f2 are packed as a (bf16,bf16) pair into one u32 word per channel:
    g[p, n] = pack(f1[p], f2[p]) on partitions 0-63, replicated to 64-127.
    One ap_gather word then carries BOTH streams of a channel, and the two
    partition halves gather DIFFERENT point chunks (all 8 gpsimd cores do
    independent work) -> 4x fewer gather-slots than the unpacked layout.
  * ap_gather's HW cost scales with num_idxs only; the source AP is passed
    as a narrow window at column 0 (the ucode addresses src by base +
    index, num_elems is the real extent) so small double-buffered gather
    tiles pipeline perfectly against the DVE k-max.
  * k-max is a 4-round pairwise-max tree on DVE in bf16 (2x mode): round 1
    into t1, rounds 2-4 in place; then pair-sum + h-add fused with the BN
    sum accumulation (scalar_tensor_tensor accum_out).
  * h = (W3-W1) f is computed in phase 2 by PE (idle there) with
    zero-padded stationaries [w31|0], [0|w31] so the two point chunks land
    on the two PSUM partition halves.
  * DMA dispatch (HWDGE) is serialized at ~625ns/transfer and blocks its
    sequencer while waiting, so loads are batched into big chunks on the
    SP queue while stores/replicas dispatch from the ACT queue (in-order
    behind their producers). Phase-2/phase-1 ordering uses tiny gpsimd
    fence reads of g plus a scheduler-only barrier instead of a strict
    all-engine barrier, so phase-2 input DMAs prefetch during phase 1.
  * The last two calls are half-size to shorten the serial DVE tree tail
    after the final gather.
"""

import numpy as np
import ml_dtypes

import concourse.bass as bass
import concourse.bacc as bacc
import concourse.mybir as mybir
import concourse.tile as tile
from concourse import bass_utils

F32 = mybir.dt.float32
BF16 = mybir.dt.bfloat16
U32 = mybir.dt.uint32
I16 = mybir.dt.int16
ALU = mybir.AluOpType
BF = ml_dtypes.bfloat16

C = 64          # channels
FULL_N = 24576  # points per cloud
FULL_B = 8      # batches == cores
K = 16          # neighbors
P = 384         # points per partition-half per gather call
BN_EPS = 1e-5


def build_kernel_a(n_cores=FULL_B, N=FULL_N):
    NCALLS = N // (2 * P)    # 32
    CALLS = [P] * (NCALLS - 1) + [P // 2, P // 2]
    assert sum(CALLS) == N // 2
    FCH = 3072               # phase-1 f-load chunk (big: DMA latency ~3us)
    RCH = 4096               # replicate chunk
    IDXCH = 4                # calls per idx DMA
    FBCH = 4                 # calls per f2 DMA
    nc = bacc.Bacc("TRN2", target_bir_lowering=False, debug=False,
                   num_devices=n_cores)

    f_d = nc.dram_tensor("f", [C, N], BF16, kind="ExternalInput")
    idxw_d = nc.dram_tensor("idxw", [128, N // 2], I16, kind="ExternalInput")
    wcat_d = nc.dram_tensor("wcat", [C, 128], BF16, kind="ExternalInput")
    wpa_d = nc.dram_tensor("wpa", [C, 128], BF16, kind="ExternalInput")
    wpb_d = nc.dram_tensor("wpb", [C, 128], BF16, kind="ExternalInput")
    x_d = nc.dram_tensor("xout", [128, N // 2], BF16, kind="ExternalOutput")
    st_d = nc.dram_tensor("stats", [128, 2], F32, kind="ExternalOutput")

    with tile.TileContext(nc) as tc:
        with (
            tc.tile_pool(name="const", bufs=1) as constp,
            tc.tile_pool(name="gpool", bufs=1) as gpool,
            tc.tile_pool(name="stat", bufs=1) as statp,
            tc.tile_pool(name="fio", bufs=3) as fio,
            tc.tile_pool(name="gath", bufs=2) as gath,
            tc.tile_pool(name="tree", bufs=1) as tree,
            tc.tile_pool(name="work", bufs=2) as work,
            tc.tile_pool(name="ps1", bufs=3, space="PSUM") as ps1,
            tc.tile_pool(name="ps2", bufs=2, space="PSUM") as ps2,
        ):
            g = gpool.tile([128, N], U32)
            ft0 = fio.tile([C, FCH], BF16, tag="ft")
            nc.sync.dma_start(ft0[:], f_d.ap()[:, 0:FCH])
            wct = constp.tile([C, 128], BF16)
            wpa = constp.tile([C, 128], BF16)
            wpb = constp.tile([C, 128], BF16)
            nc.sync.dma_start(wct[:], wcat_d.ap())
            nc.sync.dma_start(wpa[:], wpa_d.ap())
            nc.sync.dma_start(wpb[:], wpb_d.ap())

            # ---- phase 1: g[p,n] = pack(f1,f2) on the low half (DVE even
            # lane, ACT odd lane), then DMA-replicate low -> high.
            for c0 in range(0, N, FCH):
                if c0 == 0:
                    ft = ft0
                else:
                    ft = fio.tile([C, FCH], BF16, tag="ft")
                    nc.sync.dma_start(ft[:], f_d.ap()[:, c0:c0 + FCH])
                for s0 in range(0, FCH, 1024):
                    col = c0 + s0
                    gp = ps1.tile([128, 1024], F32, tag="gps")
                    nc.tensor.matmul(gp[:, 0:512], wct[:],
                                     ft[:, s0:s0 + 512],
                                     start=True, stop=True)
                    nc.tensor.matmul(gp[:, 512:1024], wct[:],
                                     ft[:, s0 + 512:s0 + 1024],
                                     start=True, stop=True)
                    gb = g[0:C, col:col + 1024].bitcast(BF16).rearrange(
                        "p (n t) -> p n t", t=2)
                    nc.vector.tensor_copy(gb[:, :, 0:1].squeeze(2),
                                          gp[0:C, :])
                    nc.scalar.copy(gb[:, :, 1:2].squeeze(2), gp[C:128, :])
                    end = col + 1024
                    # dispatch each full replicate block one chunk late so
                    # the ACT sequencer never blocks on the DVE pack sem
                    pe = end - 1024
                    if pe >= RCH and pe % RCH == 0 and pe <= N - RCH:
                        nc.scalar.dma_start(g[C:128, pe - RCH:pe],
                                            g[0:C, pe - RCH:pe])
                    if end == N - 1024:
                        nc.scalar.dma_start(g[C:128, N - RCH:N - 1024],
                                            g[0:C, N - RCH:N - 1024])
                    elif end == N:
                        nc.scalar.dma_start(g[C:128, N - 1024:N],
                                            g[0:C, N - 1024:N])

            # Order all later Pool gathers after every write to g: one tiny
            # contiguous gpsimd read at the tail of each replicate block
            # (low half covers that block's pack copies via in-order DVE/ACT,
            # high half covers the replicate DMA).
            fpts = list(range(RCH, N + 1, RCH)) + [N - 1024]
            fence = statp.tile([128, 4 * len(fpts)], U32)
            for i, r in enumerate(fpts):
                nc.gpsimd.tensor_copy(fence[:, 4 * i:4 * i + 4],
                                      g[:, r - 4:r])
            # scheduler-only fence: keeps the gathers (whose narrow source
            # APs carry no dep on most of g) behind the fence reads without
            # synthesizing semaphore waits for phase-2 input DMAs.
            tc.no_sync_barrier()

            # ---- phase 2: gather + k-max + h -> x tiles, stat partials.
            # The last two calls are half-size so the serial DVE tree tail
            # after the final gather is short.
            scol = statp.tile([128, len(CALLS)], F32)
            qcol = statp.tile([128, len(CALLS)], F32)

            c0 = 0
            for q, Pq in enumerate(CALLS):
                if c0 % (IDXCH * P) == 0:
                    it = work.tile([128, IDXCH * P], I16, tag="idx")
                    nc.sync.dma_start(
                        it[:], idxw_d.ap()[:, c0:c0 + IDXCH * P])
                if (2 * c0) % (FBCH * 2 * P) == 0:
                    f2t = work.tile([C, FBCH * 2 * P], BF16, tag="f2")
                    nc.sync.dma_start(
                        f2t[:], f_d.ap()[:, 2 * c0:2 * c0 + FBCH * 2 * P])
                iq = c0 % (IDXCH * P)
                fq = (2 * c0) % (FBCH * 2 * P)

                gt = gath.tile([128, 16 * P], U32, tag="gt")
                nc.gpsimd.ap_gather(gt[:, :16 * Pq], g[:, 0:4],
                                    it[:, iq:iq + Pq], channels=128,
                                    num_elems=N, d=1, num_idxs=16 * Pq)

                hp = ps2.tile([128, P], F32, tag="hp")
                nc.tensor.matmul(hp[:, :Pq], wpa[:], f2t[:, fq:fq + Pq],
                                 start=True, stop=False)
                nc.tensor.matmul(hp[:, :Pq], wpb[:],
                                 f2t[:, fq + Pq:fq + 2 * Pq],
                                 start=False, stop=True)

                gtb = gt[:, :16 * Pq].bitcast(BF16).rearrange(
                    "p (n k t) -> p n k t", k=K, t=2)
                t1 = tree.tile([128, P * K], BF16, tag="t1")
                t1v = t1[:, :Pq * K].rearrange(
                    "p (n k t) -> p n k t", k=8, t=2)
                nc.vector.tensor_tensor(
                    t1v, gtb[:, :, 0:8, :], gtb[:, :, 8:16, :], ALU.max)
                nc.vector.tensor_tensor(
                    t1v[:, :, 0:4, :], t1v[:, :, 0:4, :], t1v[:, :, 4:8, :],
                    ALU.max)
                nc.vector.tensor_tensor(
                    t1v[:, :, 0:2, :], t1v[:, :, 0:2, :], t1v[:, :, 2:4, :],
                    ALU.max)
                nc.vector.tensor_tensor(
                    t1v[:, :, 0:1, :], t1v[:, :, 0:1, :], t1v[:, :, 1:2, :],
                    ALU.max)
                m1 = t1v[:, :, 0, 0]
                m2 = t1v[:, :, 0, 1]
                tmp = work.tile([128, P], F32, tag="tmp")
                nc.vector.tensor_tensor(tmp[:, :Pq], m1, m2, ALU.add)
                xt = work.tile([128, P], BF16, tag="xt")
                nc.vector.scalar_tensor_tensor(
                    xt[:, :Pq], tmp[:, :Pq], 1.0, hp[:, :Pq],
                    ALU.mult, ALU.add, accum_out=scol[:, q:q + 1])
                sq = work.tile([128, P], BF16, tag="sq")
                nc.scalar.activation(
                    out=sq[:, :Pq], in_=xt[:, :Pq],
                    func=mybir.ActivationFunctionType.Square,
                    accum_out=qcol[:, q:q + 1])
                nc.scalar.dma_start(x_d.ap()[:, c0:c0 + Pq], xt[:, :Pq])
                c0 += Pq

            # ---- phase 3: per-core stat partials out ----
            pair = statp.tile([128, 2], F32)
            nc.vector.tensor_reduce(pair[:, 0:1], scol[:],
                                    axis=mybir.AxisListType.X, op=ALU.add)
            nc.vector.tensor_reduce(pair[:, 1:2], qcol[:],
                                    axis=mybir.AxisListType.X, op=ALU.add)
            nc.scalar.dma_start(st_d.ap(), pair[:])

    nc.compile()
    return nc


def build_kernel_b(n_cores=FULL_B, N=FULL_N):
    """out = x * scale[p] + bias[p]; x is the [128, N/2] stacked-chunk bf16."""
    NH = N // 2
    nc = bacc.Bacc("TRN2", target_bir_lowering=False, debug=False,
                   num_devices=n_cores)
    x_d = nc.dram_tensor("xout", [128, NH], BF16, kind="ExternalInput")
    scb_d = nc.dram_tensor("scb", [128, 2], F32, kind="ExternalInput")
    out_d = nc.dram_tensor("out", [128, NH], BF16, kind="ExternalOutput")
    CH = 4096
    with tile.TileContext(nc) as tc:
        with (
            tc.tile_pool(name="const", bufs=1) as constp,
            tc.tile_pool(name="io", bufs=4) as io,
        ):
            scb = constp.tile([128, 2], F32)
            nc.sync.dma_start(scb[:], scb_d.ap())
            for i, c0 in enumerate(range(0, NH, CH)):
                w = min(CH, NH - c0)
                t = io.tile([128, CH], BF16, tag="xin")
                nc.sync.dma_start(t[:, :w], x_d.ap()[:, c0:c0 + w])
                o = io.tile([128, CH], BF16, tag="xo")
                if i % 2 == 0:
                    nc.scalar.activation(
                        out=o[:, :w], in_=t[:, :w],
                        func=mybir.ActivationFunctionType.Identity,
                        bias=scb[:, 1:2], scale=scb[:, 0:1])
                else:
                    nc.vector.tensor_scalar(
                        out=o[:, :w], in0=t[:, :w],
                        scalar1=scb[:, 0:1], scalar2=scb[:, 1:2],
                        op0=ALU.mult, op1=ALU.add)
                nc.sync.dma_start(out_d.ap()[:, c0:c0 + w], o[:, :w])
    nc.compile()
    return nc


def prep_inputs_a(f, idx, W1, W2, W3, n_cores, N):
    w31 = (W3.astype(np.float64) - W1.astype(np.float64))
    wcat = np.vstack([W1, W2]).T.astype(BF)                  # [64, 128]
    zero = np.zeros((C, C), np.float64)
    wpa = np.hstack([w31.T, zero]).astype(BF)                # [64, 128]
    wpb = np.hstack([zero, w31.T]).astype(BF)
    calls = call_sizes(N)
    in_maps = []
    for b in range(n_cores):
        # wrapped index layout: call q occupies Pq columns; rows 0-63 get
        # chunk A (points [2c0, 2c0+Pq)) x4, rows 64-127 chunk B x4.
        ib = idx[b].astype(np.int16)
        iw = np.empty((128, N // 2), np.int16)
        c0 = 0
        for Pq in calls:
            a = ib[2 * c0:2 * c0 + Pq].T                     # [K, Pq]
            bb = ib[2 * c0 + Pq:2 * c0 + 2 * Pq].T
            iw[0:64, c0:c0 + Pq] = np.tile(a, (4, 1))
            iw[64:128, c0:c0 + Pq] = np.tile(bb, (4, 1))
            c0 += Pq
        in_maps.append({
            "f": np.ascontiguousarray(f[b]).astype(BF),
            "idxw": np.ascontiguousarray(iw),
            "wcat": np.ascontiguousarray(wcat),
            "wpa": np.ascontiguousarray(wpa),
            "wpb": np.ascontiguousarray(wpb),
        })
    return in_maps


def host_scale_bias(stats, gamma, beta, total_cnt):
    """stats: [B, 128, 2] per-core partial (sum, sumsq) -> scb [128, 2]."""
    tot = stats.astype(np.float64).sum(axis=0)     # [128, 2]
    tot = tot[0:C] + tot[C:128]                    # fold partition halves
    mean = tot[:, 0] / total_cnt
    var = tot[:, 1] / total_cnt - mean * mean
    rstd = 1.0 / np.sqrt(var + BN_EPS)
    scale = np.asarray(gamma, np.float64) * rstd
    bias = np.asarray(beta, np.float64) - mean * scale
    scb = np.stack([scale, bias], axis=1).astype(np.float32)  # [64, 2]
    return np.tile(scb, (2, 1)).astype(np.float32)            # [128, 2]


def call_sizes(N):
    ncalls = N // (2 * P)
    return [P] * (ncalls - 1) + [P // 2, P // 2]


def unshard_x(xo, N):
    """[128, N/2] stacked-chunk layout -> [64, N] channels-major."""
    x = np.empty((C, N), xo.dtype)
    c0 = 0
    for Pq in call_sizes(N):
        x[:, 2 * c0:2 * c0 + Pq] = xo[0:64, c0:c0 + Pq]
        x[:, 2 * c0 + Pq:2 * c0 + 2 * Pq] = xo[64:128, c0:c0 + Pq]
        c0 += Pq
    return np.ascontiguousarray(x)


_NC_CACHE = {}


def kernel(f, idx, W1, b1, W2, b2, W3, b3, gamma, beta):
    f = np.asarray(f)
    idx = np.asarray(idx)
    B, C_, N = f.shape
    key = (B, N)
    if key not in _NC_CACHE:
        _NC_CACHE[key] = (build_kernel_a(n_cores=B, N=N),
                          build_kernel_b(n_cores=B, N=N))
    nca, ncb = _NC_CACHE[key]
    in_maps = prep_inputs_a(f, idx, np.asarray(W1), np.asarray(W2),
                            np.asarray(W3), B, N)
    res_a = bass_utils.run_bass_kernel_spmd(nca, in_maps,
                                            core_ids=list(range(B)))
    stats = np.stack([res_a.results[b]["stats"] for b in range(B)])
    scb = host_scale_bias(stats, gamma, beta, B * N)
    in_maps_b = [{"xout": res_a.results[b]["xout"].view(BF), "scb": scb}
                 for b in range(B)]
    res_b = bass_utils.run_bass_kernel_spmd(ncb, in_maps_b,
                                            core_ids=list(range(B)))
    out = np.stack([unshard_x(res_b.results[b]["out"].view(BF), N)
                    for b in range(B)], axis=0)
    kernel.last_results = (res_a, res_b)
    return out.astype(np.float32)


# revision 25
# speedup vs baseline: 1.0205x; 1.0015x over previous
"""Trainium2 Bass kernels for DGCNN-style edge-conv block (gnn_message_passing).

Math (per batch b):
  f1 = W1 f; f2 = W2 f  (biases provably cancel: per-channel constants pass
      through the k-max and are removed by train-mode BN)
  x  = max_k f1[:, idx[n,k]] + max_k f2[:, idx[n,k]] + (W3 - W1) f
  out = BN(x) over (B, N) per channel, gamma/beta affine.

Sharding: data-parallel over B (1 point cloud per core, 8 cores).

Cross-core BN stats: the device collective hangs under the axon PJRT path,
so this uses two launches: kernel A produces x (bf16) and per-core [128,2]
partial stats; the host reduces the tiny stat tensors and computes
scale/bias; kernel B applies the affine normalization.

Kernel A layout tricks (all verified on HW):
  * f1/f2 are packed as a (bf16,bf16) pair into one u32 word per channel:
    g[p, n] = pack(f1[p], f2[p]) on partitions 0-63, replicated to 64-127.
    One ap_gather word then carries BOTH streams of a channel, and the two
    partition halves gather DIFFERENT point chunks (all 8 gpsimd cores do
    independent work) -> 4x fewer gather-slots than the unpacked layout.
  * ap_gather's HW cost scales with num_idxs only; the source AP is passed
    as a narrow window at column 0 (the ucode addresses src by base +
    index, num_elems is the real extent) so small double-buffered gather
    tiles pipeline perfectly against the DVE k-max.
  * k-max is a 4-round pairwise-max tree on DVE in bf16 (2x mode): round 1
    into t1, rounds 2-4 in place; then pair-sum + h-add fused with the BN
    sum accumulation (scalar_tensor_tensor accum_out).
  * h = (W3-W1) f is computed in phase 2 by PE (idle there) with
    zero-padded stationaries [w31|0], [0|w31] so the two point chunks land
    on the two PSUM partition halves.
  * DMA dispatch (HWDGE) is serialized at ~625ns/transfer and blocks its
    sequencer while waiting, so loads are batched into big chunks on the
    SP queue while stores/replicas dispatch from the ACT queue (in-order
    behind their producers). Phase-2/phase-1 ordering uses tiny gpsimd
    fence reads of g plus a scheduler-only barrier instead of a strict
    all-engine barrier, so phase-2 input DMAs prefetch during phase 1.
  * The last two calls are half-size to shorten the serial DVE tree tail
    after the final gather.
"""

import numpy as np
import ml_dtypes

import concourse.bass as bass
import concourse.bacc as bacc
import concourse.mybir as mybir
import concourse.tile as tile
from concourse import bass_utils

F32 = mybir.dt.float32
BF16 = mybir.dt.bfloat16
U32 = mybir.dt.uint32
I16 = mybir.dt.int16
ALU = mybir.AluOpType
BF = ml_dtypes.bfloat16

C = 64          # channels
FULL_N = 24576  # points per cloud
FULL_B = 8      # batches == cores
K = 16          # neighbors
P = 384         # points per partition-half per gather call
BN_EPS = 1e-5


def build_kernel_a(n_cores=FULL_B, N=FULL_N):
    NCALLS = N // (2 * P)    # 32
    CALLS = [P] * (NCALLS - 1) + [P // 2, P // 2]
    assert sum(CALLS) == N // 2
    FCH = 3072               # phase-1 f-load chunk (big: DMA latency ~3us)
    RCH = 4096               # replicate chunk
    IDXCH = 4                # calls per idx DMA
    FBCH = 4                 # calls per f2 DMA
    nc = bacc.Bacc("TRN2", target_bir_lowering=False, debug=False,
                   num_devices=n_cores)

    f_d = nc.dram_tensor("f", [C, N], BF16, kind="ExternalInput")
    idxw_d = nc.dram_tensor("idxw", [128, N // 2], I16, kind="ExternalInput")
    wcat_d = nc.dram_tensor("wcat", [C, 128], BF16, kind="ExternalInput")
    wpa_d = nc.dram_tensor("wpa", [C, 128], BF16, kind="ExternalInput")
    wpb_d = nc.dram_tensor("wpb", [C, 128], BF16, kind="ExternalInput")
    x_d = nc.dram_tensor("xout", [128, N // 2], BF16, kind="ExternalOutput")
    st_d = nc.dram_tensor("stats", [128, 2], F32, kind="ExternalOutput")

    with tile.TileContext(nc) as tc:
        with (
            tc.tile_pool(name="const", bufs=1) as constp,
            tc.tile_pool(name="gpool", bufs=1) as gpool,
            tc.tile_pool(name="stat", bufs=1) as statp,
            tc.tile_pool(name="fio", bufs=3) as fio,
            tc.tile_pool(name="gath", bufs=2) as gath,
            tc.tile_pool(name="tree", bufs=1) as tree,
            tc.tile_pool(name="work", bufs=2) as work,
            tc.tile_pool(name="ps1", bufs=3, space="PSUM") as ps1,
            tc.tile_pool(name="ps2", bufs=2, space="PSUM") as ps2,
        ):
            g = gpool.tile([128, N], U32)
            ft0 = fio.tile([C, FCH], BF16, tag="ft")
            nc.sync.dma_start(ft0[:], f_d.ap()[:, 0:FCH])
            wct = constp.tile([C, 128], BF16)
            wpa = constp.tile([C, 128], BF16)
            wpb = constp.tile([C, 128], BF16)
            nc.sync.dma_start(wct[:], wcat_d.ap())
            nc.sync.dma_start(wpa[:], wpa_d.ap())
            nc.sync.dma_start(wpb[:], wpb_d.ap())

            # ---- phase 1: g[p,n] = pack(f1,f2) on the low half (DVE even
            # lane, ACT odd lane), then DMA-replicate low -> high.
            for c0 in range(0, N, FCH):
                if c0 == 0:
                    ft = ft0
                else:
                    ft = fio.tile([C, FCH], BF16, tag="ft")
                    nc.sync.dma_start(ft[:], f_d.ap()[:, c0:c0 + FCH])
                for s0 in range(0, FCH, 1024):
                    col = c0 + s0
                    gp = ps1.tile([128, 1024], F32, tag="gps")
                    nc.tensor.matmul(gp[:, 0:512], wct[:],
                                     ft[:, s0:s0 + 512],
                                     start=True, stop=True)
                    nc.tensor.matmul(gp[:, 512:1024], wct[:],
                                     ft[:, s0 + 512:s0 + 1024],
                                     start=True, stop=True)
                    gb = g[0:C, col:col + 1024].bitcast(BF16).rearrange(
                        "p (n t) -> p n t", t=2)
                    nc.vector.tensor_copy(gb[:, :, 0:1].squeeze(2),
                                          gp[0:C, :])
                    nc.scalar.copy(gb[:, :, 1:2].squeeze(2), gp[C:128, :])
                    end = col + 1024
                    # dispatch each full replicate block one chunk late so
                    # the ACT sequencer never blocks on the DVE pack sem
                    pe = end - 1024
                    if pe >= RCH and pe % RCH == 0 and pe <= N - RCH:
                        nc.scalar.dma_start(g[C:128, pe - RCH:pe],
                                            g[0:C, pe - RCH:pe])
                    if end == N - 1024:
                        nc.scalar.dma_start(g[C:128, N - RCH:N - 1024],
                                            g[0:C, N - RCH:N - 1024])
                    elif end == N:
                        nc.scalar.dma_start(g[C:128, N - 1024:N],
                                            g[0:C, N - 1024:N])

            # Order all later Pool gathers after every write to g: one tiny
            # contiguous gpsimd read at the tail of each replicate block
            # (low half covers that block's pack copies via in-order DVE/ACT,
            # high half covers the replicate DMA).
            fpts = list(range(RCH, N + 1, RCH)) + [N - 1024]
            fence = statp.tile([128, 4 * len(fpts)], U32)
            for i, r in enumerate(fpts):
                nc.gpsimd.tensor_copy(fence[:, 4 * i:4 * i + 4],
                                      g[:, r - 4:r])
            # scheduler-only fence: keeps the gathers (whose narrow source
            # APs carry no dep on most of g) behind the fence reads without
            # synthesizing semaphore waits for phase-2 input DMAs.
            tc.no_sync_barrier()

            # ---- phase 2: gather + k-max + h -> x tiles, stat partials.
            # The last two calls are half-size so the serial DVE tree tail
            # after the final gather is short.
            scol = statp.tile([128, len(CALLS)], F32)
            qcol = statp.tile([128, len(CALLS)], F32)

            c0 = 0
            for q, Pq in enumerate(CALLS):
                if c0 % (IDXCH * P) == 0:
                    it = work.tile([128, IDXCH * P], I16, tag="idx")
                    nc.sync.dma_start(
                        it[:], idxw_d.ap()[:, c0:c0 + IDXCH * P])
                if (2 * c0) % (FBCH * 2 * P) == 0:
                    f2t = work.tile([C, FBCH * 2 * P], BF16, tag="f2")
                    nc.sync.dma_start(
                        f2t[:], f_d.ap()[:, 2 * c0:2 * c0 + FBCH * 2 * P])
                iq = c0 % (IDXCH * P)
                fq = (2 * c0) % (FBCH * 2 * P)

                gt = gath.tile([128, 16 * P], U32, tag="gt")
                nc.gpsimd.ap_gather(gt[:, :16 * Pq], g[:, 0:4],
                                    it[:, iq:iq + Pq], channels=128,
                                    num_elems=N, d=1, num_idxs=16 * Pq)

                hp = ps2.tile([128, P], F32, tag="hp")
                nc.tensor.matmul(hp[:, :Pq], wpa[:], f2t[:, fq:fq + Pq],
                                 start=True, stop=False)
                nc.tensor.matmul(hp[:, :Pq], wpb[:],
                                 f2t[:, fq + Pq:fq + 2 * Pq],
                                 start=False, stop=True)

                gtb = gt[:, :16 * Pq].bitcast(BF16).rearrange(
                    "p (n k t) -> p n k t", k=K, t=2)
                t1 = tree.tile([128, P * K], BF16, tag="t1")
                t1v = t1[:, :Pq * K].rearrange(
                    "p (n k t) -> p n k t", k=8, t=2)
                nc.vector.tensor_tensor(
                    t1v, gtb[:, :, 0:8, :], gtb[:, :, 8:16, :], ALU.max)
                nc.vector.tensor_tensor(
                    t1v[:, :, 0:4, :], t1v[:, :, 0:4, :], t1v[:, :, 4:8, :],
                    ALU.max)
                nc.vector.tensor_tensor(
                    t1v[:, :, 0:2, :], t1v[:, :, 0:2, :], t1v[:, :, 2:4, :],
                    ALU.max)
                nc.vector.tensor_tensor(
                    t1v[:, :, 0:1, :], t1v[:, :, 0:1, :], t1v[:, :, 1:2, :],
                    ALU.max)
                m1 = t1v[:, :, 0, 0]
                m2 = t1v[:, :, 0, 1]
                tmp = work.tile([128, P], F32, tag="tmp")
                nc.vector.tensor_tensor(tmp[:, :Pq], m1, m2, ALU.add)
                xt = work.tile([128, P], BF16, tag="xt")
                nc.vector.scalar_tensor_tensor(
                    xt[:, :Pq], tmp[:, :Pq], 1.0, hp[:, :Pq],
                    ALU.mult, ALU.add, accum_out=scol[:, q:q + 1])
                sq = work.tile([128, P], BF16, tag="sq")
                nc.scalar.activation(
                    out=sq[:, :Pq], in_=xt[:, :Pq],
                    func=mybir.ActivationFunctionType.Square,
                    accum_out=qcol[:, q:q + 1])
                nc.scalar.dma_start(x_d.ap()[:, c0:c0 + Pq], xt[:, :Pq])
                c0 += Pq

            # ---- phase 3: per-core stat partials out ----
            pair = statp.tile([128, 2], F32)
            nc.vector.tensor_reduce(pair[:, 0:1], scol[:],
                                    axis=mybir.AxisListType.X, op=ALU.add)
            nc.vector.tensor_reduce(pair[:, 1:2], qcol[:],
                                    axis=mybir.AxisListType.X, op=ALU.add)
            nc.scalar.dma_start(st_d.ap(), pair[:])

    nc.compile()
    return nc


def build_kernel_b(n_cores=FULL_B, N=FULL_N):
    """out = x * scale[p] + bias[p]; x is the [128, N/2] stacked-chunk bf16."""
    NH = N // 2
    nc = bacc.Bacc("TRN2", target_bir_lowering=False, debug=False,
                   num_devices=n_cores)
    x_d = nc.dram_tensor("xout", [128, NH], BF16, kind="ExternalInput")
    scb_d = nc.dram_tensor("scb", [128, 2], F32, kind="ExternalInput")
    out_d = nc.dram_tensor("out", [128, NH], BF16, kind="ExternalOutput")
    CH = 3072
    with tile.TileContext(nc) as tc:
        with (
            tc.tile_pool(name="const", bufs=1) as constp,
            tc.tile_pool(name="io", bufs=4) as io,
        ):
            scb = constp.tile([128, 2], F32)
            nc.sync.dma_start(scb[:], scb_d.ap())
            for i, c0 in enumerate(range(0, NH, CH)):
                w = min(CH, NH - c0)
                t = io.tile([128, CH], BF16, tag="xin")
                nc.sync.dma_start(t[:, :w], x_d.ap()[:, c0:c0 + w])
                o = io.tile([128, CH], BF16, tag="xo")
                if i % 2 == 0:
                    nc.scalar.activation(
                        out=o[:, :w], in_=t[:, :w],
                        func=mybir.ActivationFunctionType.Identity,
                        bias=scb[:, 1:2], scale=scb[:, 0:1])
                else:
                    nc.vector.tensor_scalar(
                        out=o[:, :w], in0=t[:, :w],
                        scalar1=scb[:, 0:1], scalar2=scb[:, 1:2],
                        op0=ALU.mult, op1=ALU.add)
                nc.sync.dma_start(out_d.ap()[:, c0:c0 + w], o[:, :w])
    nc.compile()
    return nc


def prep_inputs_a(f, idx, W1, W2, W3, n_cores, N):
    w31 = (W3.astype(np.float64) - W1.astype(np.float64))
    wcat = np.vstack([W1, W2]).T.astype(BF)                  # [64, 128]
    zero = np.zeros((C, C), np.float64)
    wpa = np.hstack([w31.T, zero]).astype(BF)                # [64, 128]
    wpb = np.hstack([zero, w31.T]).astype(BF)
    calls = call_sizes(N)
    in_maps = []
    for b in range(n_cores):
        # wrapped index layout: call q occupies Pq columns; rows 0-63 get
        # chunk A (points [2c0, 2c0+Pq)) x4, rows 64-127 chunk B x4.
        ib = idx[b].astype(np.int16)
        iw = np.empty((128, N // 2), np.int16)
        c0 = 0
        for Pq in calls:
            a = ib[2 * c0:2 * c0 + Pq].T                     # [K, Pq]
            bb = ib[2 * c0 + Pq:2 * c0 + 2 * Pq].T
            iw[0:64, c0:c0 + Pq] = np.tile(a, (4, 1))
            iw[64:128, c0:c0 + Pq] = np.tile(bb, (4, 1))
            c0 += Pq
        in_maps.append({
            "f": np.ascontiguousarray(f[b]).astype(BF),
            "idxw": np.ascontiguousarray(iw),
            "wcat": np.ascontiguousarray(wcat),
            "wpa": np.ascontiguousarray(wpa),
            "wpb": np.ascontiguousarray(wpb),
        })
    return in_maps


def host_scale_bias(stats, gamma, beta, total_cnt):
    """stats: [B, 128, 2] per-core partial (sum, sumsq) -> scb [128, 2]."""
    tot = stats.astype(np.float64).sum(axis=0)     # [128, 2]
    tot = tot[0:C] + tot[C:128]                    # fold partition halves
    mean = tot[:, 0] / total_cnt
    var = tot[:, 1] / total_cnt - mean * mean
    rstd = 1.0 / np.sqrt(var + BN_EPS)
    scale = np.asarray(gamma, np.float64) * rstd
    bias = np.asarray(beta, np.float64) - mean * scale
    scb = np.stack([scale, bias], axis=1).astype(np.float32)  # [64, 2]
    return np.tile(scb, (2, 1)).astype(np.float32)            # [128, 2]


def call_sizes(N):
    ncalls = N // (2 * P)
    return [P] * (ncalls - 1) + [P // 2, P // 2]


def unshard_x(xo, N):
    """[128, N/2] stacked-chunk layout -> [64, N] channels-major."""
    x = np.empty((C, N), xo.dtype)
    c0 = 0
    for Pq in call_sizes(N):
        x[:, 2 * c0:2 * c0 + Pq] = xo[0:64, c0:c0 + Pq]
        x[:, 2 * c0 + Pq:2 * c0 + 2 * Pq] = xo[64:128, c0:c0 + Pq]
        c0 += Pq
    return np.ascontiguousarray(x)


_NC_CACHE = {}


def kernel(f, idx, W1, b1, W2, b2, W3, b3, gamma, beta):
    f = np.asarray(f)
    idx = np.asarray(idx)
    B, C_, N = f.shape
    key = (B, N)
    if key not in _NC_CACHE:
        _NC_CACHE[key] = (build_kernel_a(n_cores=B, N=N),
                          build_kernel_b(n_cores=B, N=N))
    nca, ncb = _NC_CACHE[key]
    in_maps = prep_inputs_a(f, idx, np.asarray(W1), np.asarray(W2),
                            np.asarray(W3), B, N)
    res_a = bass_utils.run_bass_kernel_spmd(nca, in_maps,
                                            core_ids=list(range(B)))
    stats = np.stack([res_a.results[b]["stats"] for b in range(B)])
    scb = host_scale_bias(stats, gamma, beta, B * N)
    in_maps_b = [{"xout": res_a.results[b]["xout"].view(BF), "scb": scb}
                 for b in range(B)]
    res_b = bass_utils.run_bass_kernel_spmd(ncb, in_maps_b,
                                            core_ids=list(range(B)))
    out = np.stack([unshard_x(res_b.results[b]["out"].view(BF), N)
                    for b in range(B)], axis=0)
    kernel.last_results = (res_a, res_b)
    return out.astype(np.float32)
